# revision 46
# baseline (speedup 1.0000x reference)
"""Multi-head attention (B=4, S=2048, H=16, d_model=1024, d_k=d_v=64) on 8
Trainium2 NeuronCores.

Sharding: 8 cores = 4 batches x 2 query-halves. Each core computes all 16
heads for its (batch, query-half): K/V projections are recomputed per
query-half (duplicated within a batch pair) so that no inter-core
communication is needed; outputs are disjoint and concatenated on the host.

Per-core pipeline (all matmuls fp32r, fp32 accumulate):
  phase 1: DMA X rows -> PE-transpose 128x128 tiles (batched PSUM->SBUF
           copies split across DVE and ACT) -> project v (resident, per-head
           [1|v] blocks), qT (into the persistent qt/oh-shared tiles) and
           kT (DRAM scratch, prefetched 4 pairs deep in phase 2)
  phase 2: per head-pair: scoresT = kT.T @ qT (row-tiled K=64 pairs),
           e = exp(scores/8) on ACT, o = [1|v].T @ e accumulated over s
           (row 0 = softmax denominator), early PSUM->SBUF copy, then
           approx-reciprocal + gpsimd partition-broadcast normalize and a
           DMA lane-shift into pair-stacked layout (off the critical path)
  phase 3: out = concat(heads) @ W_O accumulated over 8 pair-chunks
"""

import os
import sys

for _p in ("/opt/trn_rl_repo", "/root/.axon_site/_ro/trn_rl_repo"):
    if os.path.isdir(_p) and _p not in sys.path:
        sys.path.insert(0, _p)

import numpy as np

import concourse.bass as bass  # noqa: F401
import concourse.tile as tile
from concourse import bacc, mybir
from concourse.bass_utils import run_bass_kernel_spmd
from concourse.masks import make_identity

F32 = mybir.dt.float32
F32R = mybir.dt.float32r

B, S, DM = 4, 2048, 1024
H, D = 16, 64
QH = S // 2  # query half per core
N_CORES = 8
NP = H // 2  # head pairs
N_SC = S // 128  # kv 128-chunks
N_MO = DM // 128  # model-dim 128-chunks


def _r(ap):
    return ap.bitcast(F32R)


def build(n_cores=N_CORES, phases=(1, 2, 3)):
    nc = bacc.Bacc("TRN2", target_bir_lowering=False, debug=False, num_devices=n_cores)

    # X inputs declared f32r so they can feed fp32r transpose matmuls directly
    x_q = nc.dram_tensor("Qh", [QH, DM], F32R, kind="ExternalInput").ap()
    x_k = nc.dram_tensor("K", [S, DM], F32R, kind="ExternalInput").ap()
    x_v = nc.dram_tensor("V", [S, DM], F32R, kind="ExternalInput").ap()
    # host-prepped weights: [mi=128, mo=8, (pair,head,dk)=1024]
    w_q = nc.dram_tensor("WQp", [128, N_MO, H * D], F32R, kind="ExternalInput").ap()
    w_k = nc.dram_tensor("WKp", [128, N_MO, H * D], F32R, kind="ExternalInput").ap()
    w_v = nc.dram_tensor("WVp", [128, N_MO, H * D], F32R, kind="ExternalInput").ap()
    # [mi=128, hv-chunk=8, dm=1024]
    w_o = nc.dram_tensor("WOp", [128, NP, DM], F32R, kind="ExternalInput").ap()
    out = nc.dram_tensor("out", [QH, DM], F32, kind="ExternalOutput").ap()

    # DRAM scratch for projected kT (pair-stacked [2*64, s])
    kt_sc = nc.dram_tensor("kt_sc", [NP, 128, S], F32R)

    with tile.TileContext(nc) as tc:
        with tc.tile_pool(name="persist", bufs=1) as pers:
            ident_f32 = pers.tile([128, 128], F32)
            make_identity(nc, ident_f32[:])
            ident = pers.tile([128, 128], F32R)
            nc.vector.tensor_copy(ident[:], ident_f32[:])
            ones16 = pers.tile([128, H], F32)
            nc.vector.memset(ones16[:], 1.0)

            # v resident: per s-chunk block of 16 head-slots [1|v] (65 wide)
            v_all = pers.tile([128, N_SC * H * 65], F32R, tag="v_all")
            # shared per-pair [128, QH] tiles: phase 1 writes qT (pair-stacked
            # [2*64, q]); after the last scores read, the normalized heads
            # overwrite the same tiles (Tile's WAR tracking orders this).
            qtoh = [
                pers.tile([128, QH], F32R, tag=f"qtoh{p}", name=f"qtoh{p}")
                for p in range(NP)
            ]

            # ---------------- phase 1: transpose + projections ----------
            def transpose_group(xt_pool, tpsum, xload, x_in, g, width):
                """Produce XT tile [128, N_MO, width] for rows g*width..+width.

                Returns xt with xt[:, mo, :] = X[g*width:(g+1)*width,
                mo*128:(mo+1)*128].T, fp32r-rounded.
                """
                xt = xt_pool.tile([128, N_MO, width], F32R, tag="xtg", name="xtg")
                for si in range(width // 128):
                    row0 = g * width + si * 128
                    xrow = xload.tile([128, DM], F32R, tag="xrow", bufs=9, name="xrow")
                    nc.sync.dma_start(out=xrow[:], in_=x_in[row0 : row0 + 128, :])
                    for mb in range(N_MO // 4):
                        tp = tpsum.tile([128, 512], F32, tag="tp")
                        for j in range(4):
                            mo = mb * 4 + j
                            nc.tensor.transpose(
                                _r(tp[:, j * 128 : (j + 1) * 128]),
                                xrow[:, mo * 128 : (mo + 1) * 128],
                                ident[:],
                            )
                        # one batched copy: psum [128,(4,128)] -> xt[:, 4mo, si*128+...]
                        dst = _r(
                            xt[:, mb * 4 : (mb + 1) * 4, si * 128 : (si + 1) * 128]
                        )
                        srcv = tp[:].rearrange("p (j c) -> p j c", j=4)
                        if (si + mb) % 2 == 0:
                            nc.vector.tensor_copy(dst, srcv)
                        else:
                            nc.scalar.copy(dst, srcv)
                return [xt[:, mo, :] for mo in range(N_MO)]

            with (
                tc.tile_pool(name="xload", bufs=7) as xload,
                tc.tile_pool(name="xt", bufs=2) as xtp,
                tc.tile_pool(name="wproj", bufs=1) as wpool,
            ):
                # --- V phase: v_all[sc] blocks [1|v] per head ---
                with (
                    tc.tile_pool(name="tpsum1", bufs=4, space="PSUM") as tpsum,
                    tc.tile_pool(name="ppsum1", bufs=2, space="PSUM") as ppsum,
                ):
                    wv_sb = wpool.tile([128, N_MO, H * D], F32R, tag="w3")
                    for g in range(S // 512):
                        vt = transpose_group(xtp, tpsum, xload, x_v, g, 512)
                        for si in range(4):
                            sc = g * 4 + si
                            base = sc * H * 65
                            blk = v_all[:, base : base + H * 65].rearrange(
                                "p (h w) -> p h w", h=H
                            )
                            for nch in range(2):
                                pp = ppsum.tile([128, 512], F32, tag="pp", bufs=4, name="pp")
                                for mo in range(N_MO):
                                    if g == 0 and si == 0 and nch == 0:
                                        nc.sync.dma_start(
                                            out=wv_sb[:, mo], in_=w_v[:, mo]
                                        )
                                    nc.tensor.matmul(
                                        pp[:],
                                        vt[mo][:, si * 128 : (si + 1) * 128],
                                        wv_sb[:, mo, nch * 512 : (nch + 1) * 512],
                                        start=(mo == 0),
                                        stop=(mo == N_MO - 1),
                                    )
                                nc.vector.tensor_copy(
                                    blk[:, nch * 8 : (nch + 1) * 8, 1:65],
                                    pp[:].rearrange("p (h w) -> p h w", h=8),
                                )
                            nc.vector.tensor_copy(blk[:, :, 0:1], ones16[:, :, None])

                # --- K phase (to DRAM scratch) ---
                with (
                    tc.tile_pool(name="tpsumk", bufs=4, space="PSUM") as tpsum,
                    tc.tile_pool(name="ppsumk", bufs=4, space="PSUM") as ppsum,
                ):
                    wk_sb = wpool.tile([128, N_MO, H * D], F32R, tag="w3")
                    for g in range(S // 512):
                        kt_t = transpose_group(xtp, tpsum, xload, x_k, g, 512)
                        for p in range(NP):
                            pp = ppsum.tile([128, 512], F32, tag="ppk", name="ppk")
                            for mo in range(N_MO):
                                if g == 0 and p == 0:
                                    nc.sync.dma_start(
                                        out=wk_sb[:, mo], in_=w_k[:, mo]
                                    )
                                nc.tensor.matmul(
                                    pp[:],
                                    wk_sb[:, mo, p * 128 : (p + 1) * 128],
                                    kt_t[mo][:],
                                    start=(mo == 0),
                                    stop=(mo == N_MO - 1),
                                )
                            stg = xload.tile([128, 512], F32R, tag="stgk", bufs=4, name="stg")
                            nc.scalar.copy(stg[:], pp[:])
                            nc.sync.dma_start(
                                out=kt_sc.ap()[p, :, g * 512 : (g + 1) * 512],
                                in_=stg[:],
                            )

                # --- Q phase ---
                with (
                    tc.tile_pool(name="tpsumq", bufs=4, space="PSUM") as tpsum,
                    tc.tile_pool(name="ppsumq", bufs=4, space="PSUM") as ppsum,
                ):
                    wq_sb = wpool.tile([128, N_MO, H * D], F32R, tag="w3")
                    for g in range(QH // 512):
                        qt_t = transpose_group(xtp, tpsum, xload, x_q, g, 512)
                        for p in range(NP):
                            pp = ppsum.tile([128, 512], F32, tag="ppk", name="ppk")
                            for mo in range(N_MO):
                                if g == 0 and p == 0:
                                    nc.sync.dma_start(
                                        out=wq_sb[:, mo], in_=w_q[:, mo]
                                    )
                                nc.tensor.matmul(
                                    pp[:],
                                    wq_sb[:, mo, p * 128 : (p + 1) * 128],
                                    qt_t[mo][:],
                                    start=(mo == 0),
                                    stop=(mo == N_MO - 1),
                                )
                            nc.scalar.copy(
                                qtoh[p][:, g * 512 : (g + 1) * 512], pp[:]
                            )

            # ---------------- phase 2: attention per pair ----------------
            with (
                tc.tile_pool(name="ktq", bufs=3) as ktq,
                tc.tile_pool(name="spsum", bufs=1, space="PSUM") as spsum,
                tc.tile_pool(name="epool", bufs=5) as epool,
                tc.tile_pool(name="apsum", bufs=1, space="PSUM") as apsum,
                tc.tile_pool(name="npool", bufs=1) as npool,
            ):
                for p in range(NP if 2 in phases else 0):
                    kt_pair = ktq.tile([128, S], F32R, tag="ktp", bufs=4, name="ktp")
                    nc.sync.dma_start(out=kt_pair[:], in_=kt_sc.ap()[p])
                    qt_pair = qtoh[p]
                    o_ps = [
                        apsum.tile([128, QH], F32, tag=f"o{h}", name=f"o{h}")
                        for h in range(2)
                    ]
                    for sc in range(N_SC):
                        for h in range(2):
                            lo, hi = h * 64, h * 64 + 64
                            vslot = sc * H * 65 + (2 * p + h) * 65
                            sp = spsum.tile([128, QH], F32, tag=f"sp{h}")
                            for qc in range(QH // 512):
                                nc.tensor.matmul(
                                    sp[:, qc * 512 : (qc + 1) * 512],
                                    kt_pair[lo:hi, sc * 128 : (sc + 1) * 128],
                                    qt_pair[lo:hi, qc * 512 : (qc + 1) * 512],
                                    start=True,
                                    stop=True,
                                    skip_group_check=True,
                                )
                            e_sb = epool.tile([128, QH], F32R, tag=f"e{h}")
                            nc.scalar.activation(
                                e_sb[:],
                                sp[:],
                                mybir.ActivationFunctionType.Exp,
                                scale=0.125,
                            )
                            for qc in range(QH // 512):
                                nc.tensor.matmul(
                                    o_ps[h][0:65, qc * 512 : (qc + 1) * 512],
                                    v_all[:, vslot : vslot + 65],
                                    e_sb[:, qc * 512 : (qc + 1) * 512],
                                    start=(sc == 0),
                                    stop=(sc == N_SC - 1),
                                    skip_group_check=True,
                                )
                    # early copy PSUM -> SBUF so next pair's matmuls can start
                    o_sb = npool.tile([128, 2 * QH], F32, tag="o_sb")
                    for h in range(2):
                        nc.vector.tensor_copy(
                            o_sb[0:65, h * QH : (h + 1) * QH], o_ps[h][0:65, :]
                        )
                    # normalize off the critical path (reads SBUF only)
                    recip = npool.tile([128, 2 * QH], F32, tag="recip")
                    bcast = npool.tile([128, 2 * QH], F32, tag="bcast")
                    htmp = npool.tile([128, 2 * QH], F32, tag="htmp")
                    for h in range(2):
                        o = h * QH
                        nc.vector.reciprocal_approx_fast(
                            recip[0:1, o : o + QH], o_sb[0:1, o : o + QH]
                        )
                        nc.gpsimd.partition_broadcast(
                            bcast[0:65, o : o + QH], recip[0:1, o : o + QH]
                        )
                        nc.vector.tensor_mul(
                            _r(htmp[0:65, o : o + QH]),
                            o_sb[0:65, o : o + QH],
                            bcast[0:65, o : o + QH],
                        )
                        nc.sync.dma_start(
                            out=qtoh[p][h * 64 : h * 64 + 64, :],
                            in_=_r(htmp[1:65, o : o + QH]),
                        )

            # ---------------- phase 3: output projection ----------------
            with (
                tc.tile_pool(name="wo", bufs=1) as wop,
                tc.tile_pool(name="fpsum", bufs=2, space="PSUM") as fpsum,
                tc.tile_pool(name="fout", bufs=3) as fout,
            ):
                wo_sb = wop.tile([128, NP, DM], F32R)
                nc.sync.dma_start(out=wo_sb[:], in_=w_o[:])
                for qc in range(QH // 128 if 3 in phases else 0):
                    for dmc in range(DM // 512):
                        fp = fpsum.tile([128, 512], F32, tag="fp")
                        for p in range(NP):
                            nc.tensor.matmul(
                                fp[:],
                                qtoh[p][:, qc * 128 : (qc + 1) * 128],
                                wo_sb[:, p, dmc * 512 : (dmc + 1) * 512],
                                start=(p == 0),
                                stop=(p == NP - 1),
                            )
                        fo = fout.tile([128, 512], F32, tag="fo")
                        nc.vector.tensor_copy(fo[:], fp[:])
                        nc.sync.dma_start(
                            out=out[
                                qc * 128 : (qc + 1) * 128,
                                dmc * 512 : (dmc + 1) * 512,
                            ],
                            in_=fo[:],
                        )
    nc.compile()
    return nc


_NC_CACHE = {}


def _get_nc():
    if "nc" not in _NC_CACHE:
        _NC_CACHE["nc"] = build()
    return _NC_CACHE["nc"]


def _prep_w3(w):
    # [H, DM, D] -> [mi=128, mo=8, (h d)=1024]
    return np.ascontiguousarray(
        w.transpose(1, 0, 2).reshape(N_MO, 128, H * D).transpose(1, 0, 2)
    )


def _prep_wo(w):
    # [H*D=1024, DM] -> [mi=128, chunk=8, DM]
    return np.ascontiguousarray(w.reshape(NP, 128, DM).transpose(1, 0, 2))


def kernel(Q, K, V, W_Q, W_K, W_V, W_O, _trace=False):
    Q = np.asarray(Q, dtype=np.float32)
    K = np.asarray(K, dtype=np.float32)
    V = np.asarray(V, dtype=np.float32)
    wq = _prep_w3(np.asarray(W_Q, dtype=np.float32))
    wk = _prep_w3(np.asarray(W_K, dtype=np.float32))
    wv = _prep_w3(np.asarray(W_V, dtype=np.float32))
    wo = _prep_wo(np.asarray(W_O, dtype=np.float32))

    in_maps = []
    for c in range(N_CORES):
        b, half = c // 2, c % 2
        in_maps.append(
            {
                "Qh": np.ascontiguousarray(Q[b, half * QH : (half + 1) * QH]),
                "K": np.ascontiguousarray(K[b]),
                "V": np.ascontiguousarray(V[b]),
                "WQp": wq,
                "WKp": wk,
                "WVp": wv,
                "WOp": wo,
            }
        )

    nc = _get_nc()
    res = run_bass_kernel_spmd(nc, in_maps, list(range(N_CORES)), trace=_trace)
    out = np.empty((B, S, DM), dtype=np.float32)
    for c in range(N_CORES):
        b, half = c // 2, c % 2
        out[b, half * QH : (half + 1) * QH] = res.results[c]["out"]
    if _trace:
        kernel._last_results = res
    return out
